# revision 38
# baseline (speedup 1.0000x reference)
"""Trainium2 Bass kernel for the coverage-attention module (nn_Attention_14602888806957).

Math (per batch b):
    scores[l] = context[b,l,:] @ W_w + (output[b,0,:] @ W_w) + cov_feat-term + W_b
where cov_feat-term = coverage[b,l] * (cov_w @ W_w) + (cov_b @ W_w).

softmax over l is shift-invariant, so every per-batch-constant term
(output@W_w, cov_b@W_w, W_b) drops out of all three outputs.  The kernel
computes the reduced scores
    s~[l] = context[b,l,:] @ W_w + alpha * coverage[b,l],   alpha = cov_w @ W_w
whose range for this problem is ~[-3, 5], so exp() needs no max-subtraction.

Per core (data-parallel over batch, 4 batches/core), one streaming pass over
context in [128, 2, 1024] tiles (1 MiB DMAs, 2 L-rows per partition):
  - DVE scalar_tensor_tensor: row dot products  ctx @ W  (accum along free)
  - ACT exp with per-partition bias = alpha*coverage
  - PE matmul (lhsT = exp column, rhs = ctx slice): unnormalized mix
    accumulated in PSUM across the 32 L-slices of a batch
Each batch's epilogue (softmax normalization, attn/coverage writeback, mix
transpose into K-layout) is emitted after the NEXT batch's stream so the PE
never stalls on the epilogue ping-pong.  The final [4,2048]@[2048,1024]
projection is split: the output-rows half (+bias) runs mid-stream; only the
mix half remains after the last batch.  out_w loads are staged between
batches so they never delay the context stream.
"""

import sys

sys.path.insert(0, "/opt/trn_rl_repo")

import numpy as np

import concourse.bass as bass
import concourse.bacc as bacc
import concourse.tile as tile
from concourse import mybir
from concourse.bass_utils import run_bass_kernel_spmd
from concourse import bass_isa

B, L, H = 32, 2048, 1024
NCORES = 8
BPC = B // NCORES          # batches per core
NT = L // 128              # 128-row L-slices per batch
R = 2                      # L-rows per partition per ctx tile (1 MiB DMAs)
F32 = mybir.dt.float32
F32R = mybir.dt.float32r
ALU = mybir.AluOpType
ACTF = mybir.ActivationFunctionType

CTX_BUFS = 8               # context tile buffering depth (1 MiB each)

# float32r runs the PE at 1 cycle/row (vs 4 for fp32) but crashes the
# exec unit on TRN2 via the DMA-produced path; kept as an experiment flag.
USE_F32R = False


def build_nc(use_f32r=None):
    if use_f32r is None:
        use_f32r = USE_F32R
    CT = F32R if use_f32r else F32
    nc = bacc.Bacc("TRN2", target_bir_lowering=False, debug=False)

    ctx_t = nc.dram_tensor("ctx", [BPC, L, H], CT, kind="ExternalInput")
    cov_t = nc.dram_tensor("cov", [BPC, L], F32, kind="ExternalInput")
    outp_t = nc.dram_tensor("outp", [BPC, H], CT, kind="ExternalInput")
    w_t = nc.dram_tensor("w", [1, H], F32, kind="ExternalInput")
    covw_t = nc.dram_tensor("covw", [1, H], F32, kind="ExternalInput")
    outw_t = nc.dram_tensor("outw", [2 * H, H], CT, kind="ExternalInput")
    outb_t = nc.dram_tensor("outb", [1, H], F32, kind="ExternalInput")

    o_out = nc.dram_tensor("o_out", [BPC, H], F32, kind="ExternalOutput")
    o_attn = nc.dram_tensor("o_attn", [BPC, L], F32, kind="ExternalOutput")
    o_cov = nc.dram_tensor("o_cov", [BPC, L], F32, kind="ExternalOutput")

    def dram_ap(handle, offset, ap):
        if isinstance(handle, bass.AP):
            return bass.AP(
                tensor=handle.tensor, offset=handle.offset + offset, ap=ap
            )
        return bass.AP(tensor=handle.ap().tensor, offset=offset, ap=ap)

    NCH = 2 * H // 128  # 16 K-chunks of the final projection

    with tile.TileContext(nc) as tc:
        with (
            tc.tile_pool(name="const", bufs=1) as constp,
            tc.tile_pool(name="ctxp", bufs=CTX_BUFS) as ctxp,
            tc.tile_pool(name="junkp", bufs=3) as junkp,
            tc.tile_pool(name="batchp", bufs=3) as batchp,
            tc.tile_pool(name="psmix", bufs=2, space="PSUM") as psmix,
            tc.tile_pool(name="psout", bufs=1, space="PSUM") as psoutp,
            tc.tile_pool(name="pssmall", bufs=2, space="PSUM") as pssmall,
            tc.tile_pool(name="dramp", bufs=2, space="DRAM") as dramp,
        ):
            # ---- constants (small) ----
            w_row = constp.tile([1, H], F32)
            nc.sync.dma_start(out=w_row, in_=w_t[:, :])
            covw_row = constp.tile([1, H], F32)
            nc.sync.dma_start(out=covw_row, in_=covw_t[:, :])
            w_bc = constp.tile([128, H], F32)
            nc.gpsimd.partition_broadcast(w_bc, w_row)
            ones_col = constp.tile([128, 1], F32)
            nc.vector.memset(ones_col, 1.0)
            outb_sb = constp.tile([1, H], F32)
            ones_row = constp.tile([1, 128], F32)
            nc.vector.memset(ones_row, 1.0)
            # dummy exp: pull the ACT table load (~2.7us) to t=0 so it
            # overlaps the initial DMAs instead of stalling the first real exp
            actwarm = constp.tile([1, 1], F32)
            nc.scalar.activation(
                out=actwarm, in_=ones_row[0:1, 0:1], func=ACTF.Exp
            )

            # alpha = cov_w @ W_w on one partition, then POOL-broadcast
            ajunk1 = junkp.tile([128, H], F32, tag="junk")
            alpha_1 = constp.tile([1, 1], F32)
            nc.vector.scalar_tensor_tensor(
                out=ajunk1[0:1, :],
                in0=covw_row,
                scalar=1.0,
                in1=w_row,
                op0=ALU.mult,
                op1=ALU.mult,
                accum_out=alpha_1,
            )
            alpha_bc = constp.tile([128, 1], F32)
            nc.gpsimd.partition_broadcast(alpha_bc, alpha_1)

            # combined[:, c, b]: chunk c of [mix_b, output_b] (K-layout)
            combined = constp.tile([128, NCH, BPC], CT)
            outw_sb = constp.tile([128, NCH, H], CT)
            out_ps = psoutp.tile([BPC, H], F32)

            def load_outw1(c):
                # one 512 KiB chunk; interleaved into the ctx stream so the
                # context DMAs (which feed the DVE) are never starved
                nc.sync.dma_start(
                    out=outw_sb[:, c, :],
                    in_=dram_ap(
                        outw_t, c * 128 * H, [[H, 128], [1, H]]
                    ),
                )

            def proj_chunk(c, start, stop):
                nc.tensor.matmul(
                    out_ps[0:BPC, 0:512],
                    combined[:, c, :],
                    outw_sb[:, c, 0:512],
                    start=start,
                    stop=stop,
                    skip_group_check=True,
                )
                nc.tensor.matmul(
                    out_ps[0:BPC, 512:1024],
                    combined[:, c, :],
                    outw_sb[:, c, 512:1024],
                    start=start,
                    stop=stop,
                    skip_group_check=True,
                )

            state = {}

            def stream(b, extra_dmas=()):
                def ctx_dma(t):
                    ctile = ctxp.tile([128, R, H], CT, tag="ctx")
                    nc.sync.dma_start(
                        out=ctile,
                        in_=dram_ap(
                            ctx_t,
                            b * L * H + t * 128 * R * H,
                            [[R * H, 128], [H, R], [1, H]],
                        ),
                    )
                    return ctile

                # first ctx tile in flight before the small per-batch loads
                ctile0 = ctx_dma(0)
                # coverage for this batch, [128, NT]: (p, 2t+r) -> l = 256t+2p+r
                cov_all = batchp.tile([128, NT // R, R], F32, tag="cov")
                nc.sync.dma_start(
                    out=cov_all,
                    in_=dram_ap(
                        cov_t, b * L, [[R, 128], [128 * R, NT // R], [1, R]]
                    ),
                )
                cov_flat = cov_all.rearrange("p a r -> p (a r)")
                acov_all = batchp.tile([128, NT], F32, tag="acov")
                nc.scalar.mul(acov_all, cov_flat, alpha_bc)

                raw_all = batchp.tile([128, NT], F32, tag="raw")
                e_all = batchp.tile([128, NT], CT, tag="e")
                mix_ps = psmix.tile([1, H], F32, tag="mixps")

                def emit_m2(t, ctile, tmp):
                    # m2 = e0*ctx0 + tmp  (DVE fused mul-add), then the PE
                    # sums partitions with a constant ones stationary —
                    # half the fp32 columns streamed vs per-row matmuls
                    m2 = junkp.tile([128, H], F32, tag="m2")
                    nc.vector.scalar_tensor_tensor(
                        out=m2,
                        in0=ctile[:, 0, :].bitcast(F32),
                        scalar=e_all[:, t * R : t * R + 1],
                        op0=ALU.mult,
                        in1=tmp,
                        op1=ALU.add,
                    )
                    nc.tensor.matmul(
                        mix_ps[0:1, 0:512],
                        ones_col[:, 0:1],
                        m2[:, 0:512],
                        start=(t == 0),
                        stop=(t == NT // R - 1),
                    )
                    nc.tensor.matmul(
                        mix_ps[0:1, 512:1024],
                        ones_col[:, 0:1],
                        m2[:, 512:1024],
                        start=(t == 0),
                        stop=(t == NT // R - 1),
                    )

                pending = None
                for t in range(NT // R):
                    ctile = ctile0 if t == 0 else ctx_dma(t)
                    if t % 2 == 1 and extra_dmas:
                        extra_dmas.pop(0)()
                    for r in range(R):
                        col = t * R + r
                        junk = junkp.tile([128, H], F32, tag="junk")
                        nc.vector.scalar_tensor_tensor(
                            out=junk,
                            in0=ctile[:, r, :].bitcast(F32),
                            scalar=1.0,
                            in1=w_bc,
                            op0=ALU.mult,
                            op1=ALU.mult,
                            accum_out=raw_all[:, col : col + 1],
                        )
                        nc.scalar.activation(
                            out=e_all[:, col : col + 1],
                            in_=raw_all[:, col : col + 1],
                            func=ACTF.Exp,
                            bias=acov_all[:, col : col + 1],
                            scale=1.0,
                        )
                    # tmp = e1*ctx1 on ACT right away (ACT is otherwise idle)
                    tmp = junkp.tile([128, H], F32, tag="tmp")
                    nc.scalar.mul(
                        tmp,
                        ctile[:, 1, :].bitcast(F32),
                        e_all[:, t * R + 1 : t * R + 2],
                    )
                    # DVE/PE part of the combine one tile later so the DVE
                    # never waits on this tile's ACT chain
                    if pending is not None:
                        emit_m2(*pending)
                    pending = (t, ctile, tmp)
                emit_m2(*pending)
                state[b] = (cov_flat, e_all, mix_ps)

            def epi(b, keep_pe=False):
                cov_flat, e_all, mix_ps = state.pop(b)
                # Z = sum(e): free-dim reduce on DVE, partition reduce on POOL
                esum = batchp.tile([128, 1], F32, tag="esum")
                nc.vector.tensor_reduce(
                    out=esum,
                    in_=e_all[:, :].bitcast(F32),
                    axis=mybir.AxisListType.X,
                    op=ALU.add,
                )
                zbc = batchp.tile([128, 1], F32, tag="zbc")
                nc.gpsimd.partition_all_reduce(
                    zbc, esum, channels=128, reduce_op=bass_isa.ReduceOp.add
                )
                rz_bc = batchp.tile([128, 1], F32, tag="rzbc")
                nc.vector.reciprocal(rz_bc, zbc)

                attn_sb = batchp.tile([128, NT], F32, tag="attn")
                nc.vector.tensor_scalar_mul(attn_sb, e_all[:, :].bitcast(F32), rz_bc)
                covnew = batchp.tile([128, NT], F32, tag="covnew")
                nc.vector.tensor_add(covnew, attn_sb, cov_flat)
                out_pat = [[R, 128], [128 * R, NT // R], [1, R]]
                nc.sync.dma_start(
                    out=dram_ap(o_attn, b * L, out_pat),
                    in_=attn_sb.rearrange("p (a r) -> p a r", r=R),
                )
                nc.sync.dma_start(
                    out=dram_ap(o_cov, b * L, out_pat),
                    in_=covnew.rearrange("p (a r) -> p a r", r=R),
                )

                # normalized mix: PSUM -> SBUF with the 1/Z scale fused
                mix_sb = batchp.tile([1, H], F32, tag="mixsb")
                nc.vector.tensor_scalar_mul(mix_sb, mix_ps, rz_bc[0:1, :])
                if keep_pe:
                    return mix_sb
                # partition-scatter into combined[:, 0:8, b] via a DRAM bounce
                mixscr = dramp.tile([1, H], F32, tag="mixscr")
                nc.sync.dma_start(out=mixscr, in_=mix_sb)
                nc.sync.dma_start(
                    out=combined[:, 0 : H // 128, b : b + 1],
                    in_=dram_ap(mixscr, 0, [[1, 128], [128, H // 128], [1, 1]]),
                )
                return None

            def staged_proj_out_half():
                # output-rows half of the projection + bias, early
                nc.sync.dma_start(out=outb_sb, in_=outb_t[:, :])
                for i, c in enumerate(range(H // 128, NCH)):
                    proj_chunk(c, start=(i == 0), stop=False)
                nc.tensor.matmul(
                    out_ps[0:BPC, 0:512],
                    ones_row[0:1, 0:BPC],
                    outb_sb[0:1, 0:512],
                    start=False,
                    stop=False,
                    skip_group_check=True,
                )
                nc.tensor.matmul(
                    out_ps[0:BPC, 512:1024],
                    ones_row[0:1, 0:BPC],
                    outb_sb[0:1, 512:1024],
                    start=False,
                    stop=False,
                    skip_group_check=True,
                )

            # schedule: epilogue(b) emitted after stream(b+1); out_w loads and
            # the output-half projection staged between batches
            stream(0, [lambda c=c: load_outw1(c) for c in range(8, 12)])
            # output rows -> combined[:, 8:16, b] (needed by the staged proj)
            for b in range(BPC):
                nc.sync.dma_start(
                    out=combined[:, H // 128 : NCH, b : b + 1],
                    in_=dram_ap(outp_t, b * H, [[1, 128], [128, H // 128], [1, 1]]),
                )
            stream(1, [lambda c=c: load_outw1(c) for c in range(12, 16)])
            epi(0)
            staged_proj_out_half()
            stream(2, [lambda c=c: load_outw1(c) for c in range(0, 4)])
            epi(1)
            stream(3)
            # chunks 4-7 queue AFTER the last ctx tile: the tail chain starts
            # from that tile, and these aren't read until ~14us into the tail
            for c in range(4, 8):
                load_outw1(c)
            epi(2)
            mix_sb3 = epi(3, keep_pe=True)

            # ---- remaining (mix) half of the projection + tanh ----
            # batch 3's mix transposed on the PE first, then all projection
            # matmuls back-to-back (keeps the PE ramped at full clock)
            for c in range(0, H // 128):
                tps = pssmall.tile([128, 1], F32, tag="small")
                nc.tensor.matmul(
                    tps,
                    mix_sb3[0:1, c * 128 : (c + 1) * 128],
                    ones_row[0:1, 0:1],
                    start=True,
                    stop=True,
                )
                nc.vector.tensor_copy(combined[:, c, 3:4], tps)
            for c in range(0, H // 128):
                proj_chunk(c, start=False, stop=(c == H // 128 - 1))
            final_sb = constp.tile([BPC, H], F32)
            nc.scalar.activation(
                out=final_sb[:, 0:512], in_=out_ps[0:BPC, 0:512], func=ACTF.Tanh
            )
            nc.scalar.activation(
                out=final_sb[:, 512:1024], in_=out_ps[0:BPC, 512:1024], func=ACTF.Tanh
            )
            nc.sync.dma_start(out=o_out[:, :], in_=final_sb)

    nc.compile()
    return nc


_NC = None


def kernel(output, context, coverage, W_w, W_b, cov_w, cov_b, out_w, out_b,
           _want_results_obj=False, _trace=False):
    global _NC
    if _NC is None:
        _NC = build_nc()
    nc = _NC

    output = np.ascontiguousarray(np.asarray(output, dtype=np.float32))
    context = np.ascontiguousarray(np.asarray(context, dtype=np.float32))
    coverage = np.ascontiguousarray(np.asarray(coverage, dtype=np.float32))
    w = np.asarray(W_w, dtype=np.float32).reshape(1, H)
    covw = np.asarray(cov_w, dtype=np.float32).reshape(1, H)
    outw = np.ascontiguousarray(np.asarray(out_w, dtype=np.float32).reshape(2 * H, H))
    outb = np.asarray(out_b, dtype=np.float32).reshape(1, H)

    in_maps = []
    for c in range(NCORES):
        sl = slice(c * BPC, (c + 1) * BPC)
        in_maps.append(
            {
                "ctx": np.ascontiguousarray(context[sl]),
                "cov": np.ascontiguousarray(coverage[sl]),
                "outp": np.ascontiguousarray(output[sl, 0, :]),
                "w": w,
                "covw": covw,
                "outw": outw,
                "outb": outb,
            }
        )

    res = run_bass_kernel_spmd(nc, in_maps, core_ids=list(range(NCORES)), trace=_trace)
    results = res.results

    out_full = np.concatenate([results[c]["o_out"] for c in range(NCORES)], axis=0)
    attn_full = np.concatenate([results[c]["o_attn"] for c in range(NCORES)], axis=0)
    cov_full = np.concatenate([results[c]["o_cov"] for c in range(NCORES)], axis=0)

    out_tuple = (
        out_full.reshape(B, 1, H).astype(np.float32),
        attn_full.reshape(B, 1, L).astype(np.float32),
        cov_full.reshape(B, L).astype(np.float32),
    )
    if _want_results_obj:
        return out_tuple, res
    return out_tuple


# revision 45
# speedup vs baseline: 1.0003x; 1.0003x over previous
"""Trainium2 Bass kernel for the coverage-attention module (nn_Attention_14602888806957).

Math (per batch b):
    scores[l] = context[b,l,:] @ W_w + (output[b,0,:] @ W_w) + cov_feat-term + W_b
where cov_feat-term = coverage[b,l] * (cov_w @ W_w) + (cov_b @ W_w).

softmax over l is shift-invariant, so every per-batch-constant term
(output@W_w, cov_b@W_w, W_b) drops out of all three outputs.  The kernel
computes the reduced scores
    s~[l] = context[b,l,:] @ W_w + alpha * coverage[b,l],   alpha = cov_w @ W_w
whose range for this problem is ~[-3, 5], so exp() needs no max-subtraction.

Per core (data-parallel over batch, 4 batches/core), one streaming pass over
context in [128, 2, 1024] tiles (1 MiB DMAs, 2 L-rows per partition):
  - DVE scalar_tensor_tensor: row dot products  ctx @ W  (accum along free)
  - ACT exp with per-partition bias = alpha*coverage
  - PE matmul (lhsT = exp column, rhs = ctx slice): unnormalized mix
    accumulated in PSUM across the 32 L-slices of a batch
Each batch's epilogue (softmax normalization, attn/coverage writeback, mix
transpose into K-layout) is emitted after the NEXT batch's stream so the PE
never stalls on the epilogue ping-pong.  The final [4,2048]@[2048,1024]
projection is split: the output-rows half (+bias) runs mid-stream; only the
mix half remains after the last batch.  out_w loads are staged between
batches so they never delay the context stream.
"""

import sys

sys.path.insert(0, "/opt/trn_rl_repo")

import numpy as np

import concourse.bass as bass
import concourse.bacc as bacc
import concourse.tile as tile
from concourse import mybir
from concourse.bass_utils import run_bass_kernel_spmd
from concourse import bass_isa

B, L, H = 32, 2048, 1024
NCORES = 8
BPC = B // NCORES          # batches per core
NT = L // 128              # 128-row L-slices per batch
R = 2                      # L-rows per partition per ctx tile (1 MiB DMAs)
F32 = mybir.dt.float32
F32R = mybir.dt.float32r
ALU = mybir.AluOpType
ACTF = mybir.ActivationFunctionType

CTX_BUFS = 8               # context tile buffering depth (1 MiB each)

# float32r runs the PE at 1 cycle/row (vs 4 for fp32) but crashes the
# exec unit on TRN2 via the DMA-produced path; kept as an experiment flag.
USE_F32R = False


def build_nc(use_f32r=None):
    if use_f32r is None:
        use_f32r = USE_F32R
    CT = F32R if use_f32r else F32
    nc = bacc.Bacc("TRN2", target_bir_lowering=False, debug=False)

    ctx_t = nc.dram_tensor("ctx", [BPC, L, H], CT, kind="ExternalInput")
    cov_t = nc.dram_tensor("cov", [BPC, L], F32, kind="ExternalInput")
    outp_t = nc.dram_tensor("outp", [BPC, H], CT, kind="ExternalInput")
    w_t = nc.dram_tensor("w", [1, H], F32, kind="ExternalInput")
    covw_t = nc.dram_tensor("covw", [1, H], F32, kind="ExternalInput")
    outw_t = nc.dram_tensor("outw", [2 * H, H], CT, kind="ExternalInput")
    outb_t = nc.dram_tensor("outb", [1, H], F32, kind="ExternalInput")

    o_out = nc.dram_tensor("o_out", [BPC, H], F32, kind="ExternalOutput")
    o_attn = nc.dram_tensor("o_attn", [BPC, L], F32, kind="ExternalOutput")
    o_cov = nc.dram_tensor("o_cov", [BPC, L], F32, kind="ExternalOutput")

    def dram_ap(handle, offset, ap):
        if isinstance(handle, bass.AP):
            return bass.AP(
                tensor=handle.tensor, offset=handle.offset + offset, ap=ap
            )
        return bass.AP(tensor=handle.ap().tensor, offset=offset, ap=ap)

    NCH = 2 * H // 128  # 16 K-chunks of the final projection

    with tile.TileContext(nc) as tc:
        with (
            tc.tile_pool(name="const", bufs=1) as constp,
            tc.tile_pool(name="ctxp", bufs=CTX_BUFS) as ctxp,
            tc.tile_pool(name="junkp", bufs=3) as junkp,
            tc.tile_pool(name="batchp", bufs=3) as batchp,
            tc.tile_pool(name="psmix", bufs=2, space="PSUM") as psmix,
            tc.tile_pool(name="psout", bufs=1, space="PSUM") as psoutp,
            tc.tile_pool(name="pssmall", bufs=2, space="PSUM") as pssmall,
            tc.tile_pool(name="dramp", bufs=2, space="DRAM") as dramp,
        ):
            # ---- constants (small) ----
            w_row = constp.tile([1, H], F32)
            nc.sync.dma_start(out=w_row, in_=w_t[:, :])
            covw_row = constp.tile([1, H], F32)
            nc.sync.dma_start(out=covw_row, in_=covw_t[:, :])
            w_bc = constp.tile([128, H], F32)
            nc.gpsimd.partition_broadcast(w_bc, w_row)
            ones_col = constp.tile([128, 1], F32)
            nc.vector.memset(ones_col, 1.0)
            outb_sb = constp.tile([1, H], F32)
            ones_row = constp.tile([1, 128], F32)
            nc.vector.memset(ones_row, 1.0)
            # dummy exp: pull the ACT table load (~2.7us) to t=0 so it
            # overlaps the initial DMAs instead of stalling the first real exp
            actwarm = constp.tile([1, 1], F32)
            nc.scalar.activation(
                out=actwarm, in_=ones_row[0:1, 0:1], func=ACTF.Exp
            )

            # alpha = cov_w @ W_w on one partition, then POOL-broadcast
            ajunk1 = junkp.tile([128, H], F32, tag="junk")
            alpha_1 = constp.tile([1, 1], F32)
            nc.vector.scalar_tensor_tensor(
                out=ajunk1[0:1, :],
                in0=covw_row,
                scalar=1.0,
                in1=w_row,
                op0=ALU.mult,
                op1=ALU.mult,
                accum_out=alpha_1,
            )
            alpha_bc = constp.tile([128, 1], F32)
            nc.gpsimd.partition_broadcast(alpha_bc, alpha_1)

            # combined[:, c, b]: chunk c of [mix_b, output_b] (K-layout)
            combined = constp.tile([128, NCH, BPC], CT)
            outw_sb = constp.tile([128, NCH, H], CT)
            out_ps = psoutp.tile([BPC, H], F32)

            def load_outw1(c):
                # one 512 KiB chunk; interleaved into the ctx stream so the
                # context DMAs (which feed the DVE) are never starved
                nc.sync.dma_start(
                    out=outw_sb[:, c, :],
                    in_=dram_ap(
                        outw_t, c * 128 * H, [[H, 128], [1, H]]
                    ),
                )

            def proj_chunk(c, start, stop):
                nc.tensor.matmul(
                    out_ps[0:BPC, 0:512],
                    combined[:, c, :],
                    outw_sb[:, c, 0:512],
                    start=start,
                    stop=stop,
                    skip_group_check=True,
                )
                nc.tensor.matmul(
                    out_ps[0:BPC, 512:1024],
                    combined[:, c, :],
                    outw_sb[:, c, 512:1024],
                    start=start,
                    stop=stop,
                    skip_group_check=True,
                )

            state = {}

            def stream(b, extra_dmas=()):
                def ctx_dma(t):
                    ctile = ctxp.tile([128, R, H], CT, tag="ctx")
                    nc.sync.dma_start(
                        out=ctile,
                        in_=dram_ap(
                            ctx_t,
                            b * L * H + t * 128 * R * H,
                            [[R * H, 128], [H, R], [1, H]],
                        ),
                    )
                    return ctile

                # first ctx tile in flight before the small per-batch loads
                ctile0 = ctx_dma(0)
                # coverage for this batch, [128, NT]: (p, 2t+r) -> l = 256t+2p+r
                cov_all = batchp.tile([128, NT // R, R], F32, tag="cov")
                nc.sync.dma_start(
                    out=cov_all,
                    in_=dram_ap(
                        cov_t, b * L, [[R, 128], [128 * R, NT // R], [1, R]]
                    ),
                )
                cov_flat = cov_all.rearrange("p a r -> p (a r)")
                acov_all = batchp.tile([128, NT], F32, tag="acov")
                nc.scalar.mul(acov_all, cov_flat, alpha_bc)

                raw_all = batchp.tile([128, NT], F32, tag="raw")
                e_all = batchp.tile([128, NT], CT, tag="e")
                mix_ps = psmix.tile([1, H], F32, tag="mixps")

                def emit_m2(t, ctile, tmp):
                    # m2 = e0*ctx0 + tmp  (DVE fused mul-add), then the PE
                    # sums partitions with a constant ones stationary —
                    # half the fp32 columns streamed vs per-row matmuls
                    m2 = junkp.tile([128, H], F32, tag="m2")
                    nc.vector.scalar_tensor_tensor(
                        out=m2,
                        in0=ctile[:, 0, :].bitcast(F32),
                        scalar=e_all[:, t * R : t * R + 1],
                        op0=ALU.mult,
                        in1=tmp,
                        op1=ALU.add,
                    )
                    nc.tensor.matmul(
                        mix_ps[0:1, 0:512],
                        ones_col[:, 0:1],
                        m2[:, 0:512],
                        start=(t == 0),
                        stop=(t == NT // R - 1),
                    )
                    nc.tensor.matmul(
                        mix_ps[0:1, 512:1024],
                        ones_col[:, 0:1],
                        m2[:, 512:1024],
                        start=(t == 0),
                        stop=(t == NT // R - 1),
                    )

                pending = None
                for t in range(NT // R):
                    ctile = ctile0 if t == 0 else ctx_dma(t)
                    if t % 2 == 1 and extra_dmas:
                        extra_dmas.pop(0)()
                    for r in range(R):
                        col = t * R + r
                        junk = junkp.tile([128, H], F32, tag="junk")
                        nc.vector.scalar_tensor_tensor(
                            out=junk,
                            in0=ctile[:, r, :].bitcast(F32),
                            scalar=1.0,
                            in1=w_bc,
                            op0=ALU.mult,
                            op1=ALU.mult,
                            accum_out=raw_all[:, col : col + 1],
                        )
                        nc.scalar.activation(
                            out=e_all[:, col : col + 1],
                            in_=raw_all[:, col : col + 1],
                            func=ACTF.Exp,
                            bias=acov_all[:, col : col + 1],
                            scale=1.0,
                        )
                    # tmp = e1*ctx1 on ACT right away (ACT is otherwise idle)
                    tmp = junkp.tile([128, H], F32, tag="tmp")
                    nc.scalar.mul(
                        tmp,
                        ctile[:, 1, :].bitcast(F32),
                        e_all[:, t * R + 1 : t * R + 2],
                    )
                    # DVE/PE part of the combine one tile later so the DVE
                    # never waits on this tile's ACT chain
                    if pending is not None:
                        emit_m2(*pending)
                    pending = (t, ctile, tmp)
                emit_m2(*pending)
                state[b] = (cov_flat, e_all, mix_ps)

            def epi(b, keep_pe=False):
                cov_flat, e_all, mix_ps = state.pop(b)
                # Z = sum(e): free-dim reduce on DVE, partition reduce on POOL
                esum = batchp.tile([128, 1], F32, tag="esum")
                nc.vector.tensor_reduce(
                    out=esum,
                    in_=e_all[:, :].bitcast(F32),
                    axis=mybir.AxisListType.X,
                    op=ALU.add,
                )
                zbc = batchp.tile([128, 1], F32, tag="zbc")
                nc.gpsimd.partition_all_reduce(
                    zbc, esum, channels=128, reduce_op=bass_isa.ReduceOp.add
                )
                rz_bc = batchp.tile([128, 1], F32, tag="rzbc")
                nc.vector.reciprocal(rz_bc, zbc)

                attn_sb = batchp.tile([128, NT], F32, tag="attn")
                nc.vector.tensor_scalar_mul(attn_sb, e_all[:, :].bitcast(F32), rz_bc)
                covnew = batchp.tile([128, NT], F32, tag="covnew")
                nc.vector.tensor_add(covnew, attn_sb, cov_flat)
                out_pat = [[R, 128], [128 * R, NT // R], [1, R]]
                nc.sync.dma_start(
                    out=dram_ap(o_attn, b * L, out_pat),
                    in_=attn_sb.rearrange("p (a r) -> p a r", r=R),
                )
                nc.sync.dma_start(
                    out=dram_ap(o_cov, b * L, out_pat),
                    in_=covnew.rearrange("p (a r) -> p a r", r=R),
                )

                # normalized mix: PSUM -> SBUF with the 1/Z scale fused
                mix_sb = batchp.tile([1, H], F32, tag="mixsb")
                nc.vector.tensor_scalar_mul(mix_sb, mix_ps, rz_bc[0:1, :])
                if keep_pe:
                    return mix_sb
                # partition-scatter into combined[:, 0:8, b] via a DRAM bounce
                mixscr = dramp.tile([1, H], F32, tag="mixscr")
                nc.sync.dma_start(out=mixscr, in_=mix_sb)
                nc.sync.dma_start(
                    out=combined[:, 0 : H // 128, b : b + 1],
                    in_=dram_ap(mixscr, 0, [[1, 128], [128, H // 128], [1, 1]]),
                )
                return None

            def staged_proj_out_half():
                # output-rows half of the projection + bias, early
                nc.sync.dma_start(out=outb_sb, in_=outb_t[:, :])
                for i, c in enumerate(range(H // 128, NCH)):
                    proj_chunk(c, start=(i == 0), stop=False)
                nc.tensor.matmul(
                    out_ps[0:BPC, 0:512],
                    ones_row[0:1, 0:BPC],
                    outb_sb[0:1, 0:512],
                    start=False,
                    stop=False,
                    skip_group_check=True,
                )
                nc.tensor.matmul(
                    out_ps[0:BPC, 512:1024],
                    ones_row[0:1, 0:BPC],
                    outb_sb[0:1, 512:1024],
                    start=False,
                    stop=False,
                    skip_group_check=True,
                )

            # schedule: epilogue(b) emitted after stream(b+1); out_w loads and
            # the output-half projection staged between batches
            stream(0, [lambda c=c: load_outw1(c) for c in range(8, 12)])
            # output rows -> combined[:, 8:16, b] (needed by the staged proj)
            for b in range(BPC):
                nc.sync.dma_start(
                    out=combined[:, H // 128 : NCH, b : b + 1],
                    in_=dram_ap(outp_t, b * H, [[1, 128], [128, H // 128], [1, 1]]),
                )
            stream(1, [lambda c=c: load_outw1(c) for c in range(12, 16)])
            epi(0)
            staged_proj_out_half()
            stream(2, [lambda c=c: load_outw1(c) for c in range(0, 4)])
            epi(1)
            stream(3)
            # chunks 4-7 queue AFTER the last ctx tile: the tail chain starts
            # from that tile, and these aren't read until ~14us into the tail
            for c in range(4, 8):
                load_outw1(c)
            epi(2)
            mix_sb3 = epi(3, keep_pe=True)

            # ---- remaining (mix) half of the projection + tanh ----
            # batch 3's mix transposed on the PE first (8 dense matmuls into
            # one PSUM tile, one copy out), then all projection matmuls
            # back-to-back (keeps the PE ramped at full clock)
            tps8 = pssmall.tile([128, H // 128, 1], F32, tag="small")
            for c in range(0, H // 128):
                nc.tensor.matmul(
                    tps8[:, c, :],
                    mix_sb3[0:1, c * 128 : (c + 1) * 128],
                    ones_row[0:1, 0:1],
                    start=True,
                    stop=True,
                    skip_group_check=True,
                )
            nc.vector.tensor_copy(combined[:, 0 : H // 128, 3:4], tps8)
            for c in range(0, H // 128):
                proj_chunk(c, start=False, stop=(c == H // 128 - 1))
            final_sb = constp.tile([BPC, H], F32)
            nc.scalar.activation(
                out=final_sb[:, 0:512], in_=out_ps[0:BPC, 0:512], func=ACTF.Tanh
            )
            nc.scalar.activation(
                out=final_sb[:, 512:1024], in_=out_ps[0:BPC, 512:1024], func=ACTF.Tanh
            )
            nc.sync.dma_start(out=o_out[:, :], in_=final_sb)

    nc.compile()
    return nc


_NC = None


def kernel(output, context, coverage, W_w, W_b, cov_w, cov_b, out_w, out_b,
           _want_results_obj=False, _trace=False):
    global _NC
    if _NC is None:
        _NC = build_nc()
    nc = _NC

    output = np.ascontiguousarray(np.asarray(output, dtype=np.float32))
    context = np.ascontiguousarray(np.asarray(context, dtype=np.float32))
    coverage = np.ascontiguousarray(np.asarray(coverage, dtype=np.float32))
    w = np.asarray(W_w, dtype=np.float32).reshape(1, H)
    covw = np.asarray(cov_w, dtype=np.float32).reshape(1, H)
    outw = np.ascontiguousarray(np.asarray(out_w, dtype=np.float32).reshape(2 * H, H))
    outb = np.asarray(out_b, dtype=np.float32).reshape(1, H)

    in_maps = []
    for c in range(NCORES):
        sl = slice(c * BPC, (c + 1) * BPC)
        in_maps.append(
            {
                "ctx": np.ascontiguousarray(context[sl]),
                "cov": np.ascontiguousarray(coverage[sl]),
                "outp": np.ascontiguousarray(output[sl, 0, :]),
                "w": w,
                "covw": covw,
                "outw": outw,
                "outb": outb,
            }
        )

    res = run_bass_kernel_spmd(nc, in_maps, core_ids=list(range(NCORES)), trace=_trace)
    results = res.results

    out_full = np.concatenate([results[c]["o_out"] for c in range(NCORES)], axis=0)
    attn_full = np.concatenate([results[c]["o_attn"] for c in range(NCORES)], axis=0)
    cov_full = np.concatenate([results[c]["o_cov"] for c in range(NCORES)], axis=0)

    out_tuple = (
        out_full.reshape(B, 1, H).astype(np.float32),
        attn_full.reshape(B, 1, L).astype(np.float32),
        cov_full.reshape(B, L).astype(np.float32),
    )
    if _want_results_obj:
        return out_tuple, res
    return out_tuple
